# revision 7
# baseline (speedup 1.0000x reference)
"""Trainium2 Bass kernel for ChunkAttentionMaskLayer.

Reference semantics (B=32, L=1024, N_SHIFT=10):
    out[b, i, j] = 1  iff  |i - j| <= 10  and  cumsum(mask)[b, i] == cumsum(mask)[b, j]
where cumsum is the inclusive prefix sum of chunk_st_mask along L
(equal prefix sums <=> no chunk start strictly between the two positions).

Strategy (pure data-parallel over batch, 4 batches per core):
  * chunk ids via two small matmuls (triangular-ones prefix + per-128-chunk
    carry fixup) on the PE engine, all in fp32 (exact for values <= 1024).
  * colvals[p, j] = chunk_id[j] broadcast across partitions via a
    ones[1,128]-stationary matmul into PSUM.
  * rowvals[p, t] = chunk_id[128 t + p] via one PE transpose.
  * For each 128-row stripe t, a single fused DVE op computes
      band_out = (colvals == rowval) * band_const
    over only the 148 columns [128 t - 10, 128 t + 138) that can be nonzero.
  * Only those band slices are DMAd to DRAM. Everything else stays zero:
    run_bass_kernel_spmd pre-zeroes ExternalOutput buffers (the native path
    hands np.zeros to run_neff; the axon/PJRT path donates zero buffers),
    so the untouched 99.7% of the [B, L, L] output is already zero.
"""

import numpy as np

B, L = 32, 1024
NSHIFT = 10
NCORES = 8
BPC = B // NCORES  # batches per core
T = L // 128  # 128-row stripes per batch
WB = 128 + 2 * NSHIFT  # max nonzero band width per stripe (148)

_built = None
_last_results = None  # stashed BassKernelResults for test harnesses


def _host_consts():
    f32 = np.float32
    ident32 = np.eye(32, dtype=f32)
    # ut128[c, f] = 1 if c <= f: inclusive prefix when used as matmul lhs-free
    ut128 = np.triu(np.ones((128, 128), dtype=f32))
    # bt32[q', q] = 1 if same batch-of-8 block and q' < q (strict):
    # carries[q] = sum of totals of earlier 128-chunks in the same batch
    bt32 = np.kron(np.eye(BPC, dtype=f32), np.triu(np.ones((T, T), dtype=f32), 1))
    # selbig[:, 128 q : 128 (q+1)] is the lhsT that selects row q of natq and
    # broadcasts it across all 128 output partitions: selbig[c, 128 q + p] = (c == q)
    selbig = np.kron(np.eye(BPC * T, dtype=f32), np.ones((1, 128), dtype=f32))
    # band[p, c] = 1 iff p <= c <= p + 20  (columns c index j - (128 t - 10))
    p = np.arange(128)[:, None]
    c = np.arange(WB)[None, :]
    band = ((c >= p) & (c <= p + 2 * NSHIFT)).astype(f32)
    return {
        "ident32": ident32,
        "ut128": ut128,
        "bt32": bt32,
        "selbig": selbig,
        "band": band,
    }


def _build_program():
    from contextlib import ExitStack

    import concourse.bacc as bacc
    import concourse.mybir as mybir
    import concourse.tile as tile

    f32 = mybir.dt.float32
    i32 = mybir.dt.int32
    Alu = mybir.AluOpType

    nc = bacc.Bacc(
        "TRN2", target_bir_lowering=False, debug=False, num_devices=NCORES
    )

    mask_t = nc.dram_tensor("mask", [BPC * T, 128], i32, kind="ExternalInput")
    ident32_t = nc.dram_tensor("ident32", [32, 32], f32, kind="ExternalInput")
    ut128_t = nc.dram_tensor("ut128", [128, 128], f32, kind="ExternalInput")
    bt32_t = nc.dram_tensor("bt32", [32, 32], f32, kind="ExternalInput")
    selbig_t = nc.dram_tensor("selbig", [BPC * T, BPC * T * 128], f32, kind="ExternalInput")
    band_t = nc.dram_tensor("band", [128, WB], f32, kind="ExternalInput")
    out_t = nc.dram_tensor("out", [BPC, L, L], i32, kind="ExternalOutput")
    out_ap = out_t.ap()

    with tile.TileContext(nc) as tc, ExitStack() as ctx:
        consts = ctx.enter_context(tc.tile_pool(name="consts", bufs=1))
        work = ctx.enter_context(tc.tile_pool(name="work", bufs=1))
        stp = ctx.enter_context(tc.tile_pool(name="stripes", bufs=BPC))
        cvp = ctx.enter_context(tc.tile_pool(name="cv", bufs=2, space="PSUM"))
        psm = ctx.enter_context(tc.tile_pool(name="psm", bufs=2, space="PSUM"))

        ident32 = consts.tile([32, 32], f32)
        nc.sync.dma_start(ident32[:], ident32_t.ap())
        ut128 = consts.tile([128, 128], f32)
        nc.sync.dma_start(ut128[:], ut128_t.ap())
        bt32 = consts.tile([32, 32], f32)
        nc.sync.dma_start(bt32[:], bt32_t.ap())
        selbig = consts.tile([BPC * T, BPC * T * 128], f32)
        nc.sync.dma_start(selbig[:], selbig_t.ap())
        band = consts.tile([128, WB], f32)
        nc.sync.dma_start(band[:], band_t.ap())

        # --- chunk ids for all 4 batches: natq[q, g] = cumsum[b, 128 t + g],
        # q = 8 b + t ---
        mask_i = work.tile([BPC * T, 128], i32)
        nc.sync.dma_start(mask_i[:], mask_t.ap())
        mask_f = work.tile([BPC * T, 128], f32)
        nc.vector.tensor_copy(mask_f[:], mask_i[:])

        mt_p = psm.tile([128, 32], f32, tag="sp")
        nc.tensor.transpose(mt_p[:], mask_f[:], ident32[:])
        mt_s = work.tile([128, 32], f32)
        nc.vector.tensor_copy(mt_s[:], mt_p[:])

        pcs_p = psm.tile([32, 128], f32, tag="sp")
        nc.tensor.matmul(pcs_p[:], mt_s[:], ut128[:], start=True, stop=True)
        pcs_s = work.tile([32, 128], f32)
        nc.vector.tensor_copy(pcs_s[:], pcs_p[:])

        car_p = psm.tile([32, 1], f32, tag="sp")
        nc.tensor.matmul(car_p[:], bt32[:], pcs_s[:, 127:128], start=True, stop=True)
        car_s = work.tile([32, 1], f32)
        nc.vector.tensor_copy(car_s[:], car_p[:])

        natq = work.tile([32, 128], f32)
        nc.vector.tensor_scalar_add(natq[:], pcs_s[:], car_s[:])

        natqT_p = psm.tile([128, 32], f32, tag="sp")
        nc.tensor.transpose(natqT_p[:], natq[:], ident32[:])
        natqT = work.tile([128, 32], f32)
        nc.vector.tensor_copy(natqT[:], natqT_p[:])

        # --- per batch: broadcast column ids, fused band compare, band DMA ---
        for b in range(BPC):
            st = stp.tile([128, T * WB], i32)
            cv = cvp.tile([128, L], f32)
            for t in range(T):
                q = T * b + t
                nc.tensor.matmul(
                    cv[:, 128 * t : 128 * (t + 1)],
                    selbig[:, 128 * q : 128 * (q + 1)],
                    natq[:],
                    start=True,
                    stop=True,
                )
            for t in range(T):
                c0 = max(0, 128 * t - NSHIFT)
                c1 = min(L, 128 * t + 128 + NSHIFT)
                w = c1 - c0
                z0 = NSHIFT if t == 0 else 0
                nc.vector.scalar_tensor_tensor(
                    out=st[:, WB * t : WB * t + w],
                    in0=cv[:, c0 : c0 + w],
                    scalar=natqT[:, T * b + t : T * b + t + 1],
                    in1=band[:, z0 : z0 + w],
                    op0=Alu.is_equal,
                    op1=Alu.mult,
                )
            for t in range(T):
                c0 = max(0, 128 * t - NSHIFT)
                c1 = min(L, 128 * t + 128 + NSHIFT)
                w = c1 - c0
                eng = nc.sync if t % 2 == 0 else nc.scalar
                eng.dma_start(
                    out_ap[b, 128 * t : 128 * (t + 1), c0 : c0 + w],
                    st[:, WB * t : WB * t + w],
                )

    nc.compile()
    return nc


def kernel(chunk_st_mask: np.ndarray) -> np.ndarray:
    global _built, _last_results
    from concourse.bass_utils import run_bass_kernel_spmd

    if _built is None:
        _built = _build_program()
    nc = _built

    consts = _host_consts()
    chunk_st_mask = np.asarray(chunk_st_mask)
    in_maps = []
    for k in range(NCORES):
        shard = np.ascontiguousarray(
            chunk_st_mask[k * BPC : (k + 1) * BPC], dtype=np.int32
        ).reshape(BPC * T, 128)
        in_maps.append({"mask": shard, **consts})

    res = run_bass_kernel_spmd(nc, in_maps, core_ids=list(range(NCORES)))
    _last_results = res
    outs = [res.results[k]["out"].reshape(BPC, L, L) for k in range(NCORES)]
    return np.concatenate(outs, axis=0).astype(np.int32)


# revision 12
# speedup vs baseline: 1.2905x; 1.2905x over previous
"""Trainium2 Bass kernel for ChunkAttentionMaskLayer.

Reference semantics (B=32, L=1024, N_SHIFT=10):
    out[b, i, j] = 1  iff  |i - j| <= 10  and  cumsum(mask)[b, i] == cumsum(mask)[b, j]
where cumsum is the inclusive prefix sum of chunk_st_mask along L
(equal prefix sums <=> no chunk start strictly between the two positions).

Strategy (pure data-parallel over batch, 4 batches per core):
  * chunk ids: one DVE tensor_tensor_scan over the [4, 1024] mask (one batch
    per partition), fp32 (exact, values <= 1024).
  * colvals[p, j] = chunk_id[b, j]: GpSimd partition_broadcast per batch.
  * rowvals[p, 4t+b] = chunk_id[b, 128 t + p]: eight tiny [4,128] PE transposes.
  * per 128-row stripe t, a single fused DVE scalar_tensor_tensor computes
      band_out = (colvals == rowval) * band_const
    over only the <=148 columns [128 t - 10, 128 t + 138) that can be nonzero.
  * only those band slices are DMAd out (3 DMAs per batch; the middle six
    stripes share one affine 3D access pattern). Everything else stays zero:
    run_bass_kernel_spmd pre-zeroes ExternalOutput buffers (the native path
    hands np.zeros to run_neff; the axon/PJRT path donates zero buffers),
    so the untouched 99.7% of the [B, L, L] output is already zero.
"""

import numpy as np

B, L = 32, 1024
NSHIFT = 10
NCORES = 8
BPC = B // NCORES  # batches per core
T = L // 128  # 128-row stripes per batch
WB = 128 + 2 * NSHIFT  # max nonzero band width per stripe (148)

_built = None
_last_results = None  # stashed BassKernelResults for test harnesses


def _host_consts():
    f32 = np.float32
    ident4 = np.eye(BPC, dtype=f32)
    # band[p, c] = 1 iff p <= c <= p + 20  (columns c index j - (128 t - 10))
    p = np.arange(128)[:, None]
    c = np.arange(WB)[None, :]
    band = ((c >= p) & (c <= p + 2 * NSHIFT)).astype(f32)
    return {"ident4": ident4, "band": band}


def _build_program():
    from contextlib import ExitStack

    import concourse.bacc as bacc
    import concourse.bass as bass
    import concourse.mybir as mybir
    import concourse.tile as tile

    f32 = mybir.dt.float32
    i32 = mybir.dt.int32
    Alu = mybir.AluOpType

    nc = bacc.Bacc(
        "TRN2", target_bir_lowering=False, debug=False, num_devices=NCORES
    )

    mask_t = nc.dram_tensor("mask", [BPC, L], i32, kind="ExternalInput")
    ident4_t = nc.dram_tensor("ident4", [BPC, BPC], f32, kind="ExternalInput")
    band_t = nc.dram_tensor("band", [128, WB], f32, kind="ExternalInput")
    out_t = nc.dram_tensor("out", [BPC, L, L], i32, kind="ExternalOutput")
    out_ap = out_t.ap()

    with tile.TileContext(nc) as tc, ExitStack() as ctx:
        consts = ctx.enter_context(tc.tile_pool(name="consts", bufs=1))
        work = ctx.enter_context(tc.tile_pool(name="work", bufs=1))
        stp = ctx.enter_context(tc.tile_pool(name="stripes", bufs=BPC))
        cvp = ctx.enter_context(tc.tile_pool(name="cv", bufs=2))
        psm = ctx.enter_context(tc.tile_pool(name="psm", bufs=1, space="PSUM"))

        ident4 = consts.tile([BPC, BPC], f32)
        nc.sync.dma_start(ident4[:], ident4_t.ap())
        band = consts.tile([128, WB], f32)
        nc.sync.dma_start(band[:], band_t.ap())

        # chunk ids cs[b, j] = inclusive cumsum of mask, one batch per partition
        mask_f = work.tile([BPC, L], f32)
        nc.gpsimd.dma_start(mask_f[:], mask_t.ap())  # SWDGE casts int32 -> f32
        cs = work.tile([BPC, L], f32)
        nc.vector.tensor_tensor_scan(
            cs[:], mask_f[:], mask_f[:], 0.0, op0=Alu.add, op1=Alu.bypass
        )
        # partition_broadcast reads partition 0 only: relayout the 4 batch
        # rows onto one partition (SBUF->SBUF DMAs)
        cs_flat = work.tile([1, BPC * L], f32)
        for b in range(BPC):
            nc.sync.dma_start(cs_flat[:, L * b : L * (b + 1)], cs[b : b + 1, :])

        # rowvals: natqT[p, 4 t + b] = cs[b, 128 t + p]
        natqT_p = psm.tile([128, BPC * T], f32)
        for t in range(T):
            nc.tensor.transpose(
                natqT_p[:, BPC * t : BPC * (t + 1)],
                cs[:, 128 * t : 128 * (t + 1)],
                ident4[:],
            )
        natqT = work.tile([128, BPC * T], f32)
        nc.vector.tensor_copy(natqT[:], natqT_p[:])

        for b in range(BPC):
            # colvals: cv[p, j] = cs[b, j] for every partition p
            cv = cvp.tile([128, L], f32)
            nc.gpsimd.partition_broadcast(cv[:], cs_flat[:, L * b : L * (b + 1)])

            st = stp.tile([128, T * WB], i32)
            for t in range(T):
                c0 = max(0, 128 * t - NSHIFT)
                c1 = min(L, 128 * t + 128 + NSHIFT)
                w = c1 - c0
                z0 = NSHIFT if t == 0 else 0
                nc.vector.scalar_tensor_tensor(
                    out=st[:, WB * t : WB * t + w],
                    in0=cv[:, c0 : c0 + w],
                    scalar=natqT[:, BPC * t + b : BPC * t + b + 1],
                    in1=band[:, z0 : z0 + w],
                    op0=Alu.is_equal,
                    op1=Alu.mult,
                )

            stv = st[:].rearrange("p (t c) -> p t c", t=T)
            e_edge = nc.sync if b % 2 == 0 else nc.scalar
            e_mid = nc.scalar if b % 2 == 0 else nc.sync
            # stripe 0: rows [0, 128), cols [0, 138)
            e_edge.dma_start(out_ap[b, 0:128, 0 : WB - NSHIFT], stv[:, 0, 0 : WB - NSHIFT])
            # stripes 1..6: rows [128 t + p], cols [128 t - 10 + c], one affine AP
            # offset(p, t, c) = b*L*L + (128 t + p)*L + 128 t - 10 + c
            dst_mid = bass.AP(
                out_t,
                b * L * L + (128 * L + 128) - NSHIFT,
                [[L, 128], [128 * L + 128, T - 2], [1, WB]],
            )
            e_mid.dma_start(dst_mid, stv[:, 1 : T - 1, :])
            # stripe 7: rows [896, 1024), cols [886, 1024)
            e_edge.dma_start(
                out_ap[b, 128 * (T - 1) : L, 128 * (T - 1) - NSHIFT : L],
                stv[:, T - 1, 0 : WB - NSHIFT],
            )

    nc.compile()
    return nc


def kernel(chunk_st_mask: np.ndarray) -> np.ndarray:
    global _built, _last_results
    from concourse.bass_utils import run_bass_kernel_spmd

    if _built is None:
        _built = _build_program()
    nc = _built

    consts = _host_consts()
    chunk_st_mask = np.asarray(chunk_st_mask)
    in_maps = []
    for k in range(NCORES):
        shard = np.ascontiguousarray(
            chunk_st_mask[k * BPC : (k + 1) * BPC], dtype=np.int32
        )
        in_maps.append({"mask": shard, **consts})

    res = run_bass_kernel_spmd(nc, in_maps, core_ids=list(range(NCORES)))
    _last_results = res
    outs = [res.results[k]["out"].reshape(BPC, L, L) for k in range(NCORES)]
    return np.concatenate(outs, axis=0).astype(np.int32)


# revision 18
# speedup vs baseline: 1.5775x; 1.2224x over previous
"""Trainium2 Bass kernel for ChunkAttentionMaskLayer.

Reference semantics (B=32, L=1024, N_SHIFT=10):
    out[b, i, j] = 1  iff  |i - j| <= 10  and  cumsum(mask)[b, i] == cumsum(mask)[b, j]
where cumsum is the inclusive prefix sum of chunk_st_mask along L
(equal prefix sums <=> no chunk start strictly between the two positions).

Strategy (pure data-parallel over batch, 4 batches per core), raw bacc with
hand-placed semaphores (no Tile tail barrier):
  * chunk ids: one DVE tensor_tensor_scan over the [4, 1024] int32 mask (one
    batch per partition); the DVE scan state is fp32 (exact for values <= 1024).
  * colvals cv[p, j] = cs[b, j]: fp16 selector matmuls on the PE
    (lhsT = "pick row b, broadcast to 128 partitions", rhs = cs16 chunk),
    128 columns at a time, accumulated nowhere - plain per-block matmuls into
    PSUM. fp16 is exact for the chunk-id range (<= 1024 < 2048).
  * rowvals natqT[p, 4 t + b] = cs[b, 128 t + p]: eight tiny [4, 128] PE
    transposes into PSUM; STT reads the scalar straight from PSUM.
  * per 128-row stripe t, one fused DVE scalar_tensor_tensor computes
      band_out = (cv == rowval) * band_const
    over only the <=148 columns [128 t - 10, 128 t + 138) that can be nonzero.
  * only those band slices are DMAd out (3 DMAs per batch; the middle six
    stripes share one affine 3D access pattern). Everything else stays zero:
    run_bass_kernel_spmd pre-zeroes ExternalOutput buffers (the native path
    hands np.zeros to run_neff; the axon/PJRT path donates zero buffers),
    so the untouched 99.7% of the [B, L, L] output is already zero.
"""

import numpy as np

B, L = 32, 1024
NSHIFT = 10
NCORES = 8
BPC = B // NCORES  # batches per core
T = L // 128  # 128-row stripes per batch
WB = 128 + 2 * NSHIFT  # max nonzero band width per stripe (148)

_built = None
_last_results = None  # stashed BassKernelResults for test harnesses


def _host_consts():
    f32, f16 = np.float32, np.float16
    ident4 = np.eye(BPC, dtype=f32)
    # selb4[:, 128 b : 128 (b+1)] selects row b of cs16 and broadcasts it
    # across all 128 output partitions: selb4[c, 128 b + p] = (c == b)
    selb4 = np.kron(np.eye(BPC, dtype=f16), np.ones((1, 128), dtype=f16))
    # band[p, c] = 1 iff p <= c <= p + 20  (columns c index j - (128 t - 10))
    p = np.arange(128)[:, None]
    c = np.arange(WB)[None, :]
    band = ((c >= p) & (c <= p + 2 * NSHIFT)).astype(f32)
    return {"ident4": ident4, "selb4": selb4, "band": band}


def _build_program():
    from contextlib import ExitStack

    import concourse.bacc as bacc
    import concourse.bass as bass
    import concourse.mybir as mybir

    f32 = mybir.dt.float32
    f16 = mybir.dt.float16
    i32 = mybir.dt.int32
    Alu = mybir.AluOpType

    nc = bacc.Bacc(
        "TRN2", target_bir_lowering=False, debug=False, num_devices=NCORES
    )

    mask_t = nc.dram_tensor("mask", [BPC, L], i32, kind="ExternalInput")
    ident4_t = nc.dram_tensor("ident4", [BPC, BPC], f32, kind="ExternalInput")
    selb4_t = nc.dram_tensor("selb4", [BPC, BPC * 128], f16, kind="ExternalInput")
    band_t = nc.dram_tensor("band", [128, WB], f32, kind="ExternalInput")
    out_t = nc.dram_tensor("out", [BPC, L, L], i32, kind="ExternalOutput")
    out_ap = out_t.ap()

    def edges(t):
        c0 = max(0, 128 * t - NSHIFT)
        c1 = min(L, 128 * t + 128 + NSHIFT)
        return c0, c1 - c0

    with ExitStack() as ctx:
        sb = lambda name, shape, dt: ctx.enter_context(
            nc.sbuf_tensor(name, shape, dt)
        )
        ps = lambda name, shape, dt: ctx.enter_context(
            nc.psum_tensor(name, shape, dt)
        )

        mask_i = sb("mask_i", [BPC, L], i32)
        cs = sb("cs", [BPC, L], f32)
        cs16 = sb("cs16", [BPC, L], f16)
        ident4 = sb("ident4_s", [BPC, BPC], f32)
        selb4 = sb("selb4_s", [BPC, BPC * 128], f16)
        band = sb("band_s", [128, WB], f32)
        sts = [sb(f"st{b}", [128, T * WB], i32) for b in range(BPC)]

        natqT_p = ps("natqT_p", [128, BPC * T], f32)
        cvs = [ps(f"cv{i}", [128, L], f32) for i in range(2)]

        s_in = ctx.enter_context(nc.semaphore("s_in"))
        s_in2 = ctx.enter_context(nc.semaphore("s_in2"))
        s_v = ctx.enter_context(nc.semaphore("s_v"))
        s_pe = ctx.enter_context(nc.semaphore("s_pe"))
        s_out = ctx.enter_context(nc.semaphore("s_out"))

        block = ctx.enter_context(nc.Block())

        @block.sync
        def _(sync):
            # input DMAs; mask first (critical path for the scan)
            sync.dma_start(mask_i[:], mask_t.ap()).then_inc(s_in, 16)
            sync.dma_start(ident4[:], ident4_t.ap()).then_inc(s_in2, 16)
            sync.dma_start(selb4[:], selb4_t.ap()).then_inc(s_in2, 16)
            sync.dma_start(band[:], band_t.ap()).then_inc(s_in2, 16)
            # out-DMAs for even batches
            for b in range(0, BPC, 2):
                stv = sts[b][:].rearrange("p (t c) -> p t c", t=T)
                sync.dma_start(
                    out_ap[b, 0:128, 0 : WB - NSHIFT], stv[:, 0, 0 : WB - NSHIFT]
                )._wait_ge(s_v, 3 + 8 * b).then_inc(s_out, 16)
                dst_mid = bass.AP(
                    out_t,
                    b * L * L + (128 * L + 128) - NSHIFT,
                    [[L, 128], [128 * L + 128, T - 2], [1, WB]],
                )
                sync.dma_start(dst_mid, stv[:, 1 : T - 1, :])._wait_ge(
                    s_v, 3 + 8 * b + 6
                ).then_inc(s_out, 16)
                sync.dma_start(
                    out_ap[b, 128 * (T - 1) : L, 128 * (T - 1) - NSHIFT : L],
                    stv[:, T - 1, 0 : WB - NSHIFT],
                )._wait_ge(s_v, 3 + 8 * b + 7).then_inc(s_out, 16)
            # all 12 out-DMAs complete before the NEFF may finish
            sync.wait_ge(s_out, 16 * 3 * BPC)

        @block.scalar
        def _(scalar):
            # out-DMAs for odd batches
            for b in range(1, BPC, 2):
                stv = sts[b][:].rearrange("p (t c) -> p t c", t=T)
                scalar.dma_start(
                    out_ap[b, 0:128, 0 : WB - NSHIFT], stv[:, 0, 0 : WB - NSHIFT]
                )._wait_ge(s_v, 3 + 8 * b).then_inc(s_out, 16)
                dst_mid = bass.AP(
                    out_t,
                    b * L * L + (128 * L + 128) - NSHIFT,
                    [[L, 128], [128 * L + 128, T - 2], [1, WB]],
                )
                scalar.dma_start(dst_mid, stv[:, 1 : T - 1, :])._wait_ge(
                    s_v, 3 + 8 * b + 6
                ).then_inc(s_out, 16)
                scalar.dma_start(
                    out_ap[b, 128 * (T - 1) : L, 128 * (T - 1) - NSHIFT : L],
                    stv[:, T - 1, 0 : WB - NSHIFT],
                )._wait_ge(s_v, 3 + 8 * b + 7).then_inc(s_out, 16)
            scalar.wait_ge(s_out, 16 * 3 * BPC)

        @block.tensor
        def _(tensor):
            # interleave transposes with batch-0 matmuls so the DVE STT chain
            # can start as soon as (transpose t, matmul(0, t)) are both done
            tensor.wait_ge(s_in2, 48)  # ident4 + selb4 + band loaded
            tensor.wait_ge(s_v, 2)  # cs and cs16 ready
            for t in range(T):
                nc.tensor.transpose(
                    natqT_p[:, BPC * t : BPC * (t + 1)],
                    cs[:, 128 * t : 128 * (t + 1)],
                    ident4[:],
                ).then_inc(s_pe, 1)
                nc.tensor.matmul(
                    cvs[0][:, 128 * t : 128 * (t + 1)],
                    selb4[:, 0:128],
                    cs16[:, 128 * t : 128 * (t + 1)],
                    start=True,
                    stop=True,
                ).then_inc(s_pe, 1)
            for b in range(1, BPC):
                for t in range(T):
                    mm = nc.tensor.matmul(
                        cvs[b % 2][:, 128 * t : 128 * (t + 1)],
                        selb4[:, 128 * b : 128 * (b + 1)],
                        cs16[:, 128 * t : 128 * (t + 1)],
                        start=True,
                        stop=True,
                    )
                    if b >= 2 and t == 0:
                        # cv buffer reuse: all STTs of batch b-2 must be done
                        mm._wait_ge(s_v, 10 + 8 * (b - 2))
                    mm.then_inc(s_pe, 1)

        @block.vector
        def _(vector):
            vector.wait_ge(s_in, 16)  # mask loaded
            # inclusive prefix sum; DVE converts int32 operands to fp32
            nc.vector.tensor_tensor_scan(
                cs[:], mask_i[:], mask_i[:], 0.0, op0=Alu.add, op1=Alu.bypass
            ).then_inc(s_v, 1)
            nc.vector.tensor_copy(cs16[:], cs[:])._wait_ge(s_v, 1).then_inc(s_v, 1)
            vector.wait_ge(s_in2, 48)  # band loaded
            for b in range(BPC):
                for t in range(T):
                    c0, w = edges(t)
                    z0 = NSHIFT if t == 0 else 0
                    # PSUM bank safety (PE-write + DVE-read same bank is a HW
                    # fatal): only read cv / natqT_p once every PE op that
                    # touches their banks for this batch has completed
                    need = 2 * T + T * b
                    nc.vector.scalar_tensor_tensor(
                        out=sts[b][:, WB * t : WB * t + w],
                        in0=cvs[b % 2][:, c0 : c0 + w],
                        scalar=natqT_p[:, BPC * t + b : BPC * t + b + 1],
                        in1=band[:, z0 : z0 + w],
                        op0=Alu.is_equal,
                        op1=Alu.mult,
                    )._wait_ge(s_pe, need).then_inc(s_v, 1)

    nc.compile()
    return nc


def kernel(chunk_st_mask: np.ndarray) -> np.ndarray:
    global _built, _last_results
    from concourse.bass_utils import run_bass_kernel_spmd

    if _built is None:
        _built = _build_program()
    nc = _built

    consts = _host_consts()
    chunk_st_mask = np.asarray(chunk_st_mask)
    in_maps = []
    for k in range(NCORES):
        shard = np.ascontiguousarray(
            chunk_st_mask[k * BPC : (k + 1) * BPC], dtype=np.int32
        )
        in_maps.append({"mask": shard, **consts})

    res = run_bass_kernel_spmd(nc, in_maps, core_ids=list(range(NCORES)))
    _last_results = res
    outs = [res.results[k]["out"].reshape(BPC, L, L) for k in range(NCORES)]
    return np.concatenate(outs, axis=0).astype(np.int32)
